# revision 21
# baseline (speedup 1.0000x reference)
"""Trainium2 Bass kernel for the merged multi-adapter LoRA layer.

Math (all fp32):
    t[n,b,j,d]  = sum_m x[b,j,m] * lora_A[n,d,m]
    out[n,b,j,k] = sum_d t[n,b,j,d] * lora_B[n,k,d]

Shapes: x (4,2048,4096), lora_A (4,16,4096), lora_B (4,4096,16)
        out (4,4,2048,4096)

Sharding: data-parallel over flattened tokens (b*j = 8192 -> 1024/core on
8 cores); the tiny LoRA params are replicated. Each core reads only its
16 MiB x-shard and writes its 64 MiB out-shard, so HBM traffic is minimal
(memory-bound regime).

Per-core dataflow (Tile framework):
  - x tiles [128 tok, 4096 m] are DMA'd contiguously, transposed on the
    TensorE (via identity) into [128 m, 512 tok] PSUM tiles, evacuated to
    SBUF.
  - mm1: t^T[c, tok] = sum_m A_pack[m, c] * xT[m, tok] accumulated over 32
    m-tiles; c = 32*n + d packs all 4 adapters into one matmul (columns
    16..31 of each 32-block are zero padding so mm2's lhsT/rhs partition
    bases land on 0/32/64/96).
  - mm2: out[tok, k] = sum_d t^T[32n+d, tok] * B_pack[32n+d, k]. K=16
    contraction -> the 4 adapters are packed into distinct 32-row PE
    tile_positions and run concurrently.
  - PSUM results are copied to SBUF (alternating Vector/Scalar engines)
    and DMA'd out as large contiguous stores.
"""

import numpy as np

import concourse.bacc as bacc
import concourse.bass as bass
import concourse.mybir as mybir
import concourse.tile as tile
from concourse import bass_utils
from concourse.bass import ds, ts
from concourse.masks import make_identity

F32 = mybir.dt.float32
F32R = mybir.dt.float32r  # 4-byte fp32 storage, reduced-precision 1-pass matmul

N_CORES = 8
B, J, M = 4, 2048, 4096
N, D, K = 4, 16, 4096
TOK = B * J              # 8192 flattened tokens
TOK_PER_CORE = TOK // N_CORES   # 1024
TT = 512                 # token macro-tile
N_TT = TOK_PER_CORE // TT       # 2
MT = 128                 # m (contraction) tile
N_MT = M // MT           # 32
SUB = TT // 128          # 128-token subtiles per macro-tile: 4
KT = 512                 # k tile (one PSUM bank of fp32)
OH = 2048                # k half-width per output staging tile
ADP = 32                 # partition stride per adapter in the packed dim


def build_program():
    nc = bacc.Bacc("TRN2")

    xs = nc.dram_tensor("xs", [TOK_PER_CORE, M], F32, kind="ExternalInput").ap()
    a_p = nc.dram_tensor("a_p", [128, N_MT, 128], F32R, kind="ExternalInput").ap()
    b_p = nc.dram_tensor("b_p", [128, K], F32R, kind="ExternalInput").ap()
    o = nc.dram_tensor("o", [N, TOK_PER_CORE, K], F32, kind="ExternalOutput").ap()

    QW = 1024               # x quarter-tile width (m elements)
    NQ = M // QW            # 4 quarters
    MPQ = N_MT // NQ        # 8 m-subtiles per quarter

    with tile.TileContext(nc) as tc:
        with (
            tc.tile_pool(name="const", bufs=1) as const_pool,
            tc.tile_pool(name="apool", bufs=1) as apool,
            tc.tile_pool(name="bpool", bufs=1) as bpool,
            tc.tile_pool(name="xpool", bufs=12) as xpool,
            tc.tile_pool(name="xtpool", bufs=4) as xtpool,
            tc.tile_pool(name="tpool", bufs=2) as tpool,
            tc.tile_pool(name="opool", bufs=12) as opool,
            tc.tile_pool(name="xtps", bufs=2, space="PSUM") as xtps_pool,
            tc.tile_pool(name="tps", bufs=1, space="PSUM") as tps_pool,
            tc.tile_pool(name="ops", bufs=5, space="PSUM") as ops_pool,
        ):
            ident = const_pool.tile([128, 128], F32, tag="ident")
            make_identity(nc, ident[:])

            a_sb = apool.tile([128, N_MT, 128], F32R, tag="a")
            nc.gpsimd.dma_start(a_sb[:], a_p[:])
            b_sb = bpool.tile([128, K], F32R, tag="b")
            nc.gpsimd.dma_start(b_sb[:], b_p[:])

            def emit_mm2_group(tt, s, half, t_sb):
                """mm2 + evacuate + store for one (128-token, 2048-k) block."""
                osb = [opool.tile([128, OH], F32, tag="o", name="osb") for _ in range(N)]
                # n-outer: the [16,128] lhsT stays loaded across the kt sweep
                for n in range(N):
                    for kt in range(OH // KT):
                        o_ps = ops_pool.tile([128, KT], F32, tag="ops", name="ops")
                        nc.tensor.matmul(
                            o_ps[:],
                            lhsT=t_sb[ds(ADP * n, D), ts(s, 128)],
                            rhs=b_sb[ds(ADP * n, D), ds(half * OH + kt * KT, KT)],
                            start=True,
                            stop=True,
                            tile_position=(ADP * n, 0),
                        )
                        if n % 2 == 0:
                            nc.vector.tensor_copy(osb[n][:, ts(kt, KT)], o_ps[:])
                        else:
                            nc.scalar.copy(osb[n][:, ts(kt, KT)], o_ps[:])
                    nc.sync.dma_start(
                        o[n, ds(tt * TT + s * 128, 128), ds(half * OH, OH)],
                        osb[n][:],
                    )

            # software pipeline: mm2 groups of token-tile tt-1 are emitted
            # between mm1 quarters of tile tt so stores flow during mm1
            pending = []
            for tt in range(N_TT):
                xq = {}
                for q in range(NQ):
                    for s in range(SUB):
                        xqt = xpool.tile([128, QW], F32, tag="xq", name="xq")
                        nc.gpsimd.dma_start(
                            xqt[:],
                            xs[ds(tt * TT + s * 128, 128), ds(q * QW, QW)],
                        )
                        xq[(q, s)] = xqt

                t_ps = tps_pool.tile([128, TT], F32, tag="tps", name="tps")
                for q in range(NQ):
                    for mtl in range(MPQ):
                        mt = q * MPQ + mtl
                        xt_ps = xtps_pool.tile([128, TT], F32, tag="xtps", name="xtps")
                        for s in range(SUB):
                            nc.tensor.matmul(
                                xt_ps[:, ts(s, 128)],
                                lhsT=xq[(q, s)][:, ts(mtl, 128)],
                                rhs=ident[:],
                                is_transpose=True,
                                start=(s == 0),
                                stop=(s == SUB - 1),
                            )
                        xt_sb = xtpool.tile([128, TT], F32R, tag="xt", name="xt")
                        nc.vector.tensor_copy(xt_sb[:], xt_ps[:])
                        nc.tensor.matmul(
                            t_ps[:],
                            lhsT=a_sb[:, mt, :],
                            rhs=xt_sb[:],
                            start=(mt == 0),
                            stop=(mt == N_MT - 1),
                        )
                    for _ in range(2):
                        if pending:
                            emit_mm2_group(*pending.pop(0))

                t_sb = tpool.tile([128, TT], F32R, tag="t", name="tsb")
                nc.vector.tensor_copy(t_sb[:], t_ps[:])
                for s in range(SUB):
                    for half in range(K // OH):
                        pending.append((tt, s, half, t_sb))

            for g in pending:
                emit_mm2_group(*g)

    nc.compile()
    return nc


_NC_CACHE = []


def _get_nc():
    if not _NC_CACHE:
        _NC_CACHE.append(build_program())
    return _NC_CACHE[0]


def prepare_inputs(x, lora_A, lora_B):
    x = np.ascontiguousarray(np.asarray(x, dtype=np.float32))
    lora_A = np.asarray(lora_A, dtype=np.float32)
    lora_B = np.asarray(lora_B, dtype=np.float32)

    xf = x.reshape(TOK, M)

    # a_t[m, 32n+d] = lora_A[n, d, m]; packed to [p, mt, c] so each SBUF
    # partition reads one contiguous 16 KiB row.
    a_t = np.zeros((M, 128), dtype=np.float32)
    for n in range(N):
        a_t[:, ADP * n : ADP * n + D] = lora_A[n].T
    a_pack = np.ascontiguousarray(a_t.reshape(N_MT, 128, 128).transpose(1, 0, 2))

    # b_pad[32n+d, k] = lora_B[n, k, d]
    b_pad = np.zeros((128, K), dtype=np.float32)
    for n in range(N):
        b_pad[ADP * n : ADP * n + D, :] = lora_B[n].T

    in_maps = [
        {
            "xs": np.ascontiguousarray(xf[c * TOK_PER_CORE : (c + 1) * TOK_PER_CORE]),
            "a_p": a_pack,
            "b_p": b_pad,
        }
        for c in range(N_CORES)
    ]
    return in_maps


def run(x, lora_A, lora_B, trace=False, **spmd_kwargs):
    nc = _get_nc()
    in_maps = prepare_inputs(x, lora_A, lora_B)
    res = bass_utils.run_bass_kernel_spmd(
        nc, in_maps, list(range(N_CORES)), trace=trace, **spmd_kwargs
    )
    o_full = np.concatenate([res.results[c]["o"] for c in range(N_CORES)], axis=1)
    return o_full.reshape(N, B, J, K), res


def kernel(x, lora_A, lora_B):
    out, _ = run(x, lora_A, lora_B)
    return out


# revision 22
# speedup vs baseline: 1.1396x; 1.1396x over previous
"""Trainium2 Bass kernel for the merged multi-adapter LoRA layer.

Math (all fp32):
    t[n,b,j,d]  = sum_m x[b,j,m] * lora_A[n,d,m]
    out[n,b,j,k] = sum_d t[n,b,j,d] * lora_B[n,k,d]

Shapes: x (4,2048,4096), lora_A (4,16,4096), lora_B (4,4096,16)
        out (4,4,2048,4096)

Sharding: data-parallel over flattened tokens (b*j = 8192 -> 1024/core on
8 cores); the tiny LoRA params are replicated. Each core reads only its
16 MiB x-shard and writes its 64 MiB out-shard, so HBM traffic is minimal
(memory-bound regime).

Per-core dataflow (Tile framework):
  - x tiles [128 tok, 4096 m] are DMA'd contiguously, transposed on the
    TensorE (via identity) into [128 m, 512 tok] PSUM tiles, evacuated to
    SBUF.
  - mm1: t^T[c, tok] = sum_m A_pack[m, c] * xT[m, tok] accumulated over 32
    m-tiles; c = 32*n + d packs all 4 adapters into one matmul (columns
    16..31 of each 32-block are zero padding so mm2's lhsT/rhs partition
    bases land on 0/32/64/96).
  - mm2: out[tok, k] = sum_d t^T[32n+d, tok] * B_pack[32n+d, k]. K=16
    contraction -> the 4 adapters are packed into distinct 32-row PE
    tile_positions and run concurrently.
  - PSUM results are copied to SBUF (alternating Vector/Scalar engines)
    and DMA'd out as large contiguous stores.
"""

import numpy as np

import concourse.bacc as bacc
import concourse.bass as bass
import concourse.mybir as mybir
import concourse.tile as tile
from concourse import bass_utils
from concourse.bass import ds, ts
from concourse.masks import make_identity

F32 = mybir.dt.float32
F32R = mybir.dt.float32r  # 4-byte fp32 storage, reduced-precision 1-pass matmul

N_CORES = 8
B, J, M = 4, 2048, 4096
N, D, K = 4, 16, 4096
TOK = B * J              # 8192 flattened tokens
TOK_PER_CORE = TOK // N_CORES   # 1024
TT = 512                 # token macro-tile
N_TT = TOK_PER_CORE // TT       # 2
MT = 128                 # m (contraction) tile
N_MT = M // MT           # 32
SUB = TT // 128          # 128-token subtiles per macro-tile: 4
KT = 512                 # k tile (one PSUM bank of fp32)
OH = 2048                # k half-width per output staging tile
ADP = 32                 # partition stride per adapter in the packed dim


def build_program():
    nc = bacc.Bacc("TRN2")

    xs = nc.dram_tensor("xs", [TOK_PER_CORE, M], F32, kind="ExternalInput").ap()
    a_p = nc.dram_tensor("a_p", [128, N_MT, 128], F32R, kind="ExternalInput").ap()
    b_p = nc.dram_tensor("b_p", [128, K], F32R, kind="ExternalInput").ap()
    o = nc.dram_tensor("o", [N, TOK_PER_CORE, K], F32, kind="ExternalOutput").ap()

    QW = 1024               # x quarter-tile width (m elements)
    NQ = M // QW            # 4 quarters
    MPQ = N_MT // NQ        # 8 m-subtiles per quarter

    with tile.TileContext(nc) as tc:
        with (
            tc.tile_pool(name="const", bufs=1) as const_pool,
            tc.tile_pool(name="apool", bufs=1) as apool,
            tc.tile_pool(name="bpool", bufs=1) as bpool,
            tc.tile_pool(name="xpool", bufs=12) as xpool,
            tc.tile_pool(name="xtpool", bufs=4) as xtpool,
            tc.tile_pool(name="tpool", bufs=2) as tpool,
            tc.tile_pool(name="opool", bufs=12) as opool,
            tc.tile_pool(name="xtps", bufs=2, space="PSUM") as xtps_pool,
            tc.tile_pool(name="tps", bufs=1, space="PSUM") as tps_pool,
            tc.tile_pool(name="ops", bufs=5, space="PSUM") as ops_pool,
        ):
            ident = const_pool.tile([128, 128], F32, tag="ident")
            make_identity(nc, ident[:])

            a_sb = apool.tile([128, N_MT, 128], F32R, tag="a")
            nc.gpsimd.dma_start(a_sb[:], a_p[:])
            b_sb = bpool.tile([128, K], F32R, tag="b")
            nc.gpsimd.dma_start(b_sb[:], b_p[:])

            def emit_mm2_group(tt, s, half, t_sb):
                """mm2 + evacuate + store for one (128-token, 2048-k) block."""
                osb = [opool.tile([128, OH], F32, tag="o", name="osb") for _ in range(N)]
                for kt in range(OH // KT):
                    for n in range(N):
                        o_ps = ops_pool.tile([128, KT], F32, tag="ops", name="ops")
                        nc.tensor.matmul(
                            o_ps[:],
                            lhsT=t_sb[ds(ADP * n, D), ts(s, 128)],
                            rhs=b_sb[ds(ADP * n, D), ds(half * OH + kt * KT, KT)],
                            start=True,
                            stop=True,
                            tile_position=(ADP * n, 0),
                        )
                        if n % 2 == 0:
                            nc.vector.tensor_copy(osb[n][:, ts(kt, KT)], o_ps[:])
                        else:
                            nc.scalar.copy(osb[n][:, ts(kt, KT)], o_ps[:])
                for n in range(N):
                    nc.sync.dma_start(
                        o[n, ds(tt * TT + s * 128, 128), ds(half * OH, OH)],
                        osb[n][:],
                    )

            # software pipeline: mm2 groups of token-tile tt-1 are emitted
            # between mm1 quarters of tile tt so stores flow during mm1
            pending = []
            for tt in range(N_TT):
                xq = {}
                for q in range(NQ):
                    for s in range(SUB):
                        xqt = xpool.tile([128, QW], F32, tag="xq", name="xq")
                        nc.gpsimd.dma_start(
                            xqt[:],
                            xs[ds(tt * TT + s * 128, 128), ds(q * QW, QW)],
                        )
                        xq[(q, s)] = xqt

                t_ps = tps_pool.tile([128, TT], F32, tag="tps", name="tps")
                for q in range(NQ):
                    for mtl in range(MPQ):
                        mt = q * MPQ + mtl
                        xt_ps = xtps_pool.tile([128, TT], F32, tag="xtps", name="xtps")
                        for s in range(SUB):
                            nc.tensor.matmul(
                                xt_ps[:, ts(s, 128)],
                                lhsT=xq[(q, s)][:, ts(mtl, 128)],
                                rhs=ident[:],
                                is_transpose=True,
                                start=(s == 0),
                                stop=(s == SUB - 1),
                            )
                        xt_sb = xtpool.tile([128, TT], F32R, tag="xt", name="xt")
                        nc.vector.tensor_copy(xt_sb[:], xt_ps[:])
                        nc.tensor.matmul(
                            t_ps[:],
                            lhsT=a_sb[:, mt, :],
                            rhs=xt_sb[:],
                            start=(mt == 0),
                            stop=(mt == N_MT - 1),
                        )
                    for _ in range(2):
                        if pending:
                            emit_mm2_group(*pending.pop(0))

                t_sb = tpool.tile([128, TT], F32R, tag="t", name="tsb")
                nc.vector.tensor_copy(t_sb[:], t_ps[:])
                for s in range(SUB):
                    for half in range(K // OH):
                        pending.append((tt, s, half, t_sb))

            for g in pending:
                emit_mm2_group(*g)

    nc.compile()
    return nc


_NC_CACHE = []


def _get_nc():
    if not _NC_CACHE:
        _NC_CACHE.append(build_program())
    return _NC_CACHE[0]


def prepare_inputs(x, lora_A, lora_B):
    x = np.ascontiguousarray(np.asarray(x, dtype=np.float32))
    lora_A = np.asarray(lora_A, dtype=np.float32)
    lora_B = np.asarray(lora_B, dtype=np.float32)

    xf = x.reshape(TOK, M)

    # a_t[m, 32n+d] = lora_A[n, d, m]; packed to [p, mt, c] so each SBUF
    # partition reads one contiguous 16 KiB row.
    a_t = np.zeros((M, 128), dtype=np.float32)
    for n in range(N):
        a_t[:, ADP * n : ADP * n + D] = lora_A[n].T
    a_pack = np.ascontiguousarray(a_t.reshape(N_MT, 128, 128).transpose(1, 0, 2))

    # b_pad[32n+d, k] = lora_B[n, k, d]
    b_pad = np.zeros((128, K), dtype=np.float32)
    for n in range(N):
        b_pad[ADP * n : ADP * n + D, :] = lora_B[n].T

    in_maps = [
        {
            "xs": np.ascontiguousarray(xf[c * TOK_PER_CORE : (c + 1) * TOK_PER_CORE]),
            "a_p": a_pack,
            "b_p": b_pad,
        }
        for c in range(N_CORES)
    ]
    return in_maps


def run(x, lora_A, lora_B, trace=False, **spmd_kwargs):
    nc = _get_nc()
    in_maps = prepare_inputs(x, lora_A, lora_B)
    res = bass_utils.run_bass_kernel_spmd(
        nc, in_maps, list(range(N_CORES)), trace=trace, **spmd_kwargs
    )
    o_full = np.concatenate([res.results[c]["o"] for c in range(N_CORES)], axis=1)
    return o_full.reshape(N, B, J, K), res


def kernel(x, lora_A, lora_B):
    out, _ = run(x, lora_A, lora_B)
    return out
